# revision 1
# baseline (speedup 1.0000x reference)
"""Trainium2 Bass kernel for a binarized (1w1a) BasicBlock:

    out = relu(bn2(conv2(sign(pad(relu(bn1(conv1(sign(pad(x)), sign(w1)))))), sign(w2))) + x)

with 2x3 convs, C=256, B=64, H=W=32, pad = (W: 1 left/right, H: 1 bottom).

Strategy: data-parallel over batch across 8 NeuronCores (8 images/core).
Per core each conv is an implicit GEMM: input channels on the 128 SBUF
partitions, contraction over all 256 channels in a single PE pass via fp8e4
DoubleRow matmuls (binarized values +-1/0 are exact in fp8; PSUM accumulates
fp32, so all conv sums are exact integers). Activations live in a
"shared-pad" plane layout: 33-wide rows where one zero column serves as both
the right pad of row h and the left pad of row h+1. Each image is processed
as 3 row-chunks (11/11/10 rows); each of the 6 kernel taps is one
PSUM-accumulated DoubleRow matmul per chunk whose moving operand is a 4D AP
[ci, ko, rows, 32] that walks the 33-pitch plane but streams only the 32
data columns per row - pads and the bottom pad row never hit the PE, so the
steady-state cadence is the fp8 DoubleRow roofline (1 cycle per output
pixel at 2.4GHz; ~153ns per 352-col matmul, zero stalls).

Edge scheduling (measured on HW):
 - ~3.5us of dummy warm-up matmuls sized so the HAM clock-gate reaches 8/8
   exactly when image 0's plane lands (~10.4us; two HW DMA queues at
   ~55GB/s); weights stream in tap-granular pieces timed to the matmul slot
   that needs them (tap k of a group is needed at matmul 3k).
 - the residual x streams as fp8 and the output returns as bf16 (upcast on
   host): the conv sums are exact, and these roundings contribute ~0.3% of
   the output absmax vs the 2e-2 gate, while halving DMA traffic.
 - epilogues: conv1's bn+relu+sign collapses into DVE tensor_scalar
   ((psum*inv1) is_gt -bias1 -> {0,1} fp8) written straight into conv2's
   planes (chunks j0+j1 fused into one strided op); conv2 does DVE
   scalar_tensor_tensor (psum*inv2 + x) then ACT Relu(+bias2) -> bf16.
 - outputs leave per-chunk, rotated across sync/scalar HWDGE and (mid-kernel
   only - it's ~16GB/s) gpsimd SWDGE; the last two groups run their matmuls
   chunk-outer and alternate queues strictly so the final writeback mostly
   overlaps the matmul stream.
"""

import numpy as np
import ml_dtypes

import concourse.mybir as mybir
import concourse.tile as tile
from concourse import bacc
from concourse.bass_utils import run_bass_kernel_spmd

N_CORES = 8
B, C, H, W = 64, 256, 32, 32
BL = B // N_CORES          # images per core
P = 128
KT = C // P                # channel tiles (contraction / output)
HP, WP = H + 1, W + 2      # padded spatial dims (33, 34)
NPOS = 6                   # 2x3 kernel taps
EPS = 1e-5

F32 = mybir.dt.float32
BF16 = mybir.dt.bfloat16
FP8 = mybir.dt.float8e4

# shared-pad plane layout: each padded row is 33 wide (32 data + 1 shared
# zero column that serves as row h's right pad AND row h+1's left pad), plus
# one leading zero and a zero bottom row. Conv output (h, w) lands at flat
# position h*33 + w of the PSUM chunks.
PITCH = 33
DATA0 = 1                   # leading zero (left pad of row 0)
PLANE = DATA0 + PITCH * PITCH   # 1090 = data extent incl bottom pad row
NJ = 2                      # chunks per image (16 rows = 512-col matmuls)
CH = 11 * PITCH             # 363
NPAD = 1168                 # >= PLANE + max tap offset, mult of 16
ROWS_J = (16, 16)           # valid output rows per chunk
OFF_J = (0, 528)            # chunk start offsets (plane coords)

_CACHE = {}


def _build():
    if "nc" in _CACHE:
        return _CACHE["nc"]

    nc = bacc.Bacc("TRN2", target_bir_lowering=False, debug=False)

    xq_d = nc.dram_tensor("xq1", [P, BL, KT, NPAD], FP8, kind="ExternalInput")
    x_d = nc.dram_tensor("x", [P, BL, KT, H * W], FP8, kind="ExternalInput")
    # weights laid out [mt, ci, pos, ko, co] so per-tap slices are contiguous
    w1_d = nc.dram_tensor("w1t", [KT, P, NPOS, KT, P], FP8, kind="ExternalInput")
    w2_d = nc.dram_tensor("w2t", [KT, P, NPOS, KT, P], FP8, kind="ExternalInput")
    bnv_d = nc.dram_tensor("bnv", [4, C], F32, kind="ExternalInput")
    out_d = nc.dram_tensor("out", [BL, C, H, W], BF16, kind="ExternalOutput")

    with tile.TileContext(nc) as tc:
        with (
            tc.tile_pool(name="res", bufs=1) as res,
            tc.tile_pool(name="tmp", bufs=8) as tmp,
            tc.tile_pool(name="stg", bufs=6) as stg,
            tc.tile_pool(name="psa", bufs=2, space="PSUM") as psa,
            tc.tile_pool(name="psb", bufs=3, space="PSUM") as psb,
        ):
            xq1 = [None] * BL
            xq2 = [None] * BL
            xg = [None] * BL

            def pad_memsets(q, eng):
                """Zero the pad cells of a fresh plane tile: leading zero,
                shared pad column, bottom pad row, tail."""
                v = q[:, :, DATA0:DATA0 + PITCH * PITCH].rearrange(
                    "c k (h w) -> c k h w", w=PITCH)
                eng.memset(q[:, :, 0:DATA0], 0.0)
                eng.memset(v[:, :, :, W:PITCH], 0.0)
                eng.memset(v[:, :, H:PITCH, 0:W], 0.0)
                eng.memset(q[:, :, PLANE:NPAD], 0.0)

            def interior(q, kt):
                return q[:, kt, DATA0:DATA0 + H * PITCH].rearrange(
                    "c (h w) -> c h w", w=PITCH)[:, :, 0:W]

            # ---- head DMA schedule ----
            # PE warm-up first: image 0's plane takes until ~10.4us to land
            # (two HW queues at ~60GB/s), so run ~3.5us of dummy matmuls that
            # end right then - the HAM clock-gate needs ~3.4us of sustained
            # PE activity, so the real stream starts at the full 2.4GHz with
            # no idle gap that would re-throttle it.
            wu = res.tile([P, 256], FP8, tag="wu", name="wu")
            nc.vector.memset(wu[:], 0.0)
            wups = psb.tile([P, 256], F32, tag="wups", name="wups", bufs=1)
            for _ in range(16):
                nc.tensor.matmul(wups[:], wu[:, 0:P], wu[:], start=True, stop=True)

            # Weights in tap-granular pieces (tap k of group (b, mt) is
            # needed at MM 3k, so later taps trickle in behind the first).
            # w1/mt0's tap0 rides SWDGE so neither HW queue delays it; the
            # images stream as per-ko halves down both HW queues in parallel
            # (contiguous 1168B elements - column splits would be strided and
            # halve DMA efficiency).
            w1sb, w2sb = [None, None], [None, None]
            for mt in range(KT):
                w1sb[mt] = res.tile([P, NPOS, KT, P], FP8, tag=f"w1q{mt}", name=f"w1q{mt}")
                w2sb[mt] = res.tile([P, NPOS, KT, P], FP8, tag=f"w2q{mt}", name=f"w2q{mt}")
            for b in range(BL):
                xq1[b] = res.tile([P, KT, NPAD], FP8, tag=f"xq1_{b}", name=f"xq1_{b}")

            # tap0 + bnv ride SWDGE (slow but off the critical HW queues);
            # the later taps are spread across both HW queues ordered by the
            # matmul slot that needs them
            nc.gpsimd.dma_start(w1sb[0][:, 0:1], w1_d.ap()[0][:, 0:1])
            bnsb = res.tile([P, 4 * KT], F32, tag="bnv", name="bnv")
            nc.gpsimd.dma_start(bnsb[:], bnv_d.ap().rearrange("v (t p) -> p (v t)", p=P))
            nc.sync.dma_start(xq1[0][:, 0:1], xq_d.ap()[:, 0, 0:1])
            nc.scalar.dma_start(xq1[0][:, 1:2], xq_d.ap()[:, 0, 1:2])
            nc.scalar.dma_start(w1sb[0][:, 1:2], w1_d.ap()[0][:, 1:2])
            nc.sync.dma_start(w1sb[0][:, 2:4], w1_d.ap()[0][:, 2:4])
            nc.scalar.dma_start(w1sb[0][:, 4:6], w1_d.ap()[0][:, 4:6])
            nc.sync.dma_start(w1sb[1][:, 0:1], w1_d.ap()[1][:, 0:1])
            nc.scalar.dma_start(w1sb[1][:, 1:2], w1_d.ap()[1][:, 1:2])
            nc.sync.dma_start(w1sb[1][:, 2:4], w1_d.ap()[1][:, 2:4])
            nc.scalar.dma_start(w1sb[1][:, 4:6], w1_d.ap()[1][:, 4:6])

            # remaining images: ko halves in parallel down sync + scalar
            for b in range(1, BL):
                nc.sync.dma_start(xq1[b][:, 0:1], xq_d.ap()[:, b, 0:1])
                nc.scalar.dma_start(xq1[b][:, 1:2], xq_d.ap()[:, b, 1:2])

            # w2 behind everything urgent on sync (needed ~50us in)
            nc.sync.dma_start(w2sb[0][:], w2_d.ap()[0])
            nc.sync.dma_start(w2sb[1][:], w2_d.ap()[1])

            # conv2 input planes: zero the pads (vector/gpsimd, never scalar)
            for b in range(BL):
                xq2[b] = res.tile([P, KT, NPAD], FP8, tag=f"xq2_{b}", name=f"xq2_{b}")
                pad_memsets(xq2[b], nc.vector if b % 2 else nc.gpsimd)

            # residual x (fp8: its error is ~0.2% of the output absmax): needed only by conv2's epilogue, streams in
            # behind the binarized planes
            for b in range(BL):
                xg[b] = res.tile([P, KT, H * W], FP8, tag=f"xg{b}", name=f"xg{b}")
                (nc.scalar if b % 2 else nc.sync).dma_start(xg[b][:], x_d.ap()[:, b])

            inv1sb = bnsb[:, 0 * KT:1 * KT]
            nb1sb = bnsb[:, 1 * KT:2 * KT]
            inv2sb = bnsb[:, 2 * KT:3 * KT]
            b2sb = bnsb[:, 3 * KT:4 * KT]

            def conv_groups(b, mt, wsb, src, chunk_outer=False):
                """6-tap DoubleRow accumulation for the NJ chunks of (b, mt).
                The moving operand is a 4D AP [ci, ko, rows, 32] that walks
                the 33-pitch plane but streams only the 32 data columns per
                row - the pad columns never hit the PE, so the PSUM chunks
                are dense [rows x 32] and each matmul is 3% shorter.

                Every matmul self-loads its weights (no lhsT reuse exists at
                the ISA level), so chunk order is free: pos-outer spreads a
                tap's need across 3 matmul slots (good while weights stream
                in); chunk-outer completes each chunk's PSUM ~1.7us earlier
                (good for the final group so its writeback overlaps the
                remaining matmuls)."""
                pta = psa.tile([P, 1024], F32, tag="psa", name=f"psa_{b}_{mt}")
                order = [(j, pos) for j in range(NJ) for pos in range(NPOS)] \
                    if chunk_outer else \
                    [(j, pos) for pos in range(NPOS) for j in range(NJ)]
                for j, pos in order:
                    kh, kw = divmod(pos, 3)
                    off = kh * PITCH + kw + OFF_J[j]
                    r = ROWS_J[j]
                    out = pta[:, 512 * j:512 * j + r * W]
                    nc.tensor.matmul(
                        out,
                        wsb[mt][:, pos],
                        src[:, :, off: off + r * PITCH].rearrange(
                            "c k (r x) -> c k r x", x=PITCH)[:, :, :, 0:W],
                        start=(pos == 0),
                        stop=(pos == NPOS - 1),
                        perf_mode=mybir.MatmulPerfMode.DoubleRow,
                    )
                return pta

            # ---- conv1 + binarize epilogue ----
            for b in range(BL):
                for mt in range(KT):
                    pta = conv_groups(b, mt, w1sb, xq1[b])
                    q2v = interior(xq2[b], mt)
                    # one DVE op for chunks j0+j1 (strided 4D APs), one for
                    # the 10-row j2 - DVE op count is what the tail is paced
                    # by, not element count
                    nc.vector.tensor_scalar(
                        q2v[:].rearrange("c (u r) w -> c u r w", r=16),
                        pta[:].rearrange(
                            "c (u q) -> c u q", q=512).rearrange(
                            "c u (r w) -> c u r w", w=W),
                        inv1sb[:, mt:mt + 1],
                        nb1sb[:, mt:mt + 1],
                        mybir.AluOpType.mult,
                        mybir.AluOpType.is_gt,
                    )

            # ---- conv2 + bn2 + residual + relu ----
            OUTQ = (nc.sync, nc.gpsimd, nc.scalar)
            def tail_b7():
                """b7: interleave the two groups' chunks
                (mt0-j0, mt1-j0, mt0-j1, ...) so all but the last 80KB chunk
                drains while matmuls still run. Each chunk keeps its own
                6-tap accumulation; queues alternate strictly."""
                ps_ = {}
                for mt in range(KT):
                    ps_[mt] = (psa.tile([P, 1024], F32, tag="psa", name=f"psa_7_{mt}"),
                               psb.tile([P, 512], F32, tag="psb", name=f"psb_7_{mt}"))
                def mms7(j, mt):
                    pta, ptb = ps_[mt]
                    r = ROWS_J[j]
                    out = pta[:, 512 * j:512 * j + r * W] if j < 2 \
                        else ptb[:, 0:r * W]
                    for pos in range(NPOS):
                        kh, kw = divmod(pos, 3)
                        off = kh * PITCH + kw + OFF_J[j]
                        nc.tensor.matmul(
                            out, w2sb[mt][:, pos],
                            xq2[7][:, :, off: off + r * PITCH].rearrange(
                                "c k (r x) -> c k r x", x=PITCH)[:, :, :, 0:W],
                            start=(pos == 0), stop=(pos == NPOS - 1),
                            perf_mode=mybir.MatmulPerfMode.DoubleRow,
                        )

                ot7 = {mt: stg.tile([P, H * W], BF16, tag="ot", name=f"ot_7_{mt}")
                       for mt in range(KT)}
                dst7 = {mt: out_d.ap()[7, mt * P:(mt + 1) * P].rearrange(
                    "c h w -> c (h w)") for mt in range(KT)}

                def epi01_7(mt, qs):
                    pta, _ = ps_[mt]
                    tt = tmp.tile([P, 22 * W], F32, tag="t01", name=f"t01_7_{mt}")
                    nc.vector.scalar_tensor_tensor(
                        tt[:].rearrange("c (u n) -> c u n", u=2),
                        pta[:].rearrange("c (u q) -> c u q", q=512)[:, :, 0:352],
                        inv2sb[:, mt:mt + 1],
                        xg[7][:, mt, 0:704].rearrange("c (u n) -> c u n", u=2),
                        mybir.AluOpType.mult,
                        mybir.AluOpType.add,
                    )
                    nc.scalar.activation(
                        ot7[mt][:, 0:704], tt[:],
                        mybir.ActivationFunctionType.Relu,
                        bias=b2sb[:, mt:mt + 1], scale=1.0,
                    )
                    for qi, q in enumerate(qs):
                        q.dma_start(dst7[mt][:, qi * 352:qi * 352 + 352],
                                    ot7[mt][:, qi * 352:qi * 352 + 352])

                def epi2_7(mt, qs):
                    _, ptb = ps_[mt]
                    n0, n = 22 * W, 10 * W
                    tt = tmp.tile([P, 11 * W], F32, tag="t2", name=f"t7_{mt}_2")
                    nc.vector.scalar_tensor_tensor(
                        tt[:, 0:n], ptb[:, 0:n],
                        inv2sb[:, mt:mt + 1],
                        xg[7][:, mt, n0:n0 + n],
                        mybir.AluOpType.mult,
                        mybir.AluOpType.add,
                    )
                    nc.scalar.activation(
                        ot7[mt][:, n0:n0 + n], tt[:, 0:n],
                        mybir.ActivationFunctionType.Relu,
                        bias=b2sb[:, mt:mt + 1], scale=1.0,
                    )
                    h = n // len(qs)
                    for qi, q in enumerate(qs):
                        q.dma_start(dst7[mt][:, n0 + qi * h:n0 + qi * h + h],
                                    ot7[mt][:, n0 + qi * h:n0 + qi * h + h])

                mms7(0, 0); mms7(0, 1); mms7(1, 0)
                epi01_7(0, (nc.sync, nc.scalar))
                mms7(1, 1)
                epi01_7(1, (nc.scalar, nc.sync))
                mms7(2, 0)
                epi2_7(0, (nc.sync,))
                mms7(2, 1)
                epi2_7(1, (nc.scalar, nc.sync))

            for b in range(BL):
                for mt in range(KT):
                    last = (mt == KT - 1) and (b == BL - 1)
                    tail = b == BL - 1
                    pta = conv_groups(b, mt, w2sb, xq2[b], chunk_outer=tail)
                    ot = stg.tile([P, H * W], BF16, tag="ot", name=f"ot_{b}_{mt}")
                    dst = out_d.ap()[b, mt * P:(mt + 1) * P].rearrange(
                        "c h w -> c (h w)")

                    def epi(j, r0, r1, qs, eng=None):
                        """bn2 + residual + relu + store for rows [r0, r1) of
                        chunk j, output DMA on queue(s) qs. eng=None: relu on
                        the ACT engine; else STT + add/max relu both on eng
                        (used at the tail to keep the HW DMA queues free of
                        compute)."""
                        n0, n = (16 * j + r0) * W, (r1 - r0) * W
                        tt = tmp.tile([P, 16 * W], F32, tag="t2",
                                      name=f"t2_{b}_{mt}_{j}_{r0}")
                        nc.vector.scalar_tensor_tensor(
                            tt[:, 0:n],
                            pta[:, 512 * j + r0 * W:512 * j + r1 * W],
                            inv2sb[:, mt:mt + 1],
                            xg[b][:, mt, n0:n0 + n],
                            mybir.AluOpType.mult,
                            mybir.AluOpType.add,
                        )
                        if eng is None:
                            nc.scalar.activation(
                                ot[:, n0:n0 + n], tt[:, 0:n],
                                mybir.ActivationFunctionType.Relu,
                                bias=b2sb[:, mt:mt + 1],
                                scale=1.0,
                            )
                        else:
                            # gpsimd is ~15x slower on tensor ops and can't
                            # read PSUM - eng must be nc.vector
                            eng.tensor_scalar(
                                ot[:, n0:n0 + n], tt[:, 0:n],
                                b2sb[:, mt:mt + 1], 0.0,
                                mybir.AluOpType.add,
                                mybir.AluOpType.max,
                            )
                        h = n // len(qs)
                        for qi, q in enumerate(qs):
                            q.dma_start(dst[:, n0 + qi * h:n0 + qi * h + h],
                                        ot[:, n0 + qi * h:n0 + qi * h + h])

                    if last:
                        # final group: chunk-outer matmuls, half-chunk
                        # pieces balanced across both HW queues (128KB each)
                        epi(0, 0, 8, (nc.sync,))
                        epi(0, 8, 16, (nc.scalar,))
                        epi(1, 0, 8, (nc.sync,))
                        epi(1, 8, 16, (nc.scalar,))
                    elif b == BL - 1:
                        # second-to-last group: strict queue alternation
                        epi(0, 0, 16, (nc.sync,))
                        epi(1, 0, 16, (nc.scalar,))
                    elif b == BL - 2:
                        # b6: SWDGE takes a chunk per group early enough to
                        # drain before the tail
                        epi(0, 0, 16, (nc.gpsimd,))
                        epi(1, 0, 16, (nc.sync if mt == 0 else nc.scalar,))
                    else:
                        # SWDGE transfers linger (~16GB/s), so it only gets
                        # mid-kernel chunks; head/tail groups stay on the HW
                        # queues
                        if 1 <= b <= 5:
                            qs = [OUTQ[(b * NJ + j) % 3] for j in range(NJ)]
                        elif (b + mt) % 2 == 0:
                            qs = [nc.sync, nc.scalar]
                        else:
                            qs = [nc.scalar, nc.sync]
                        for j in range(NJ):
                            epi(j, 0, ROWS_J[j], (qs[j],))

    nc.compile()
    _CACHE["nc"] = nc
    return nc


def _prep(w1, w2, gamma1, beta1, mean1, var1, gamma2, beta2, mean2, var2):
    """Host-side: fold BN, binarize + lay out weights as lhsT tiles."""
    def fold(gamma, beta, mean, var):
        inv = (gamma.astype(np.float64) / np.sqrt(var.astype(np.float64) + EPS))
        inv = inv.astype(np.float32)
        bias = (beta.astype(np.float32) - mean.astype(np.float32) * inv)
        return inv, bias

    inv1, bias1 = fold(gamma1, beta1, mean1, var1)
    inv2, bias2 = fold(gamma2, beta2, mean2, var2)

    def wt(w):
        # [O, I, 2, 3] -> lhsT layout [mt, ci, pos, ko, co]
        s = np.sign(w).astype(np.float32)
        arr = s.transpose(1, 2, 3, 0).reshape(KT, P, NPOS, KT, P)  # [ko,ci,pos,mt,co]
        arr = arr.transpose(3, 1, 2, 0, 4)
        return np.ascontiguousarray(arr).astype(mybir.dt.np(FP8))

    bnv = np.ascontiguousarray(np.stack([inv1, -bias1, inv2, bias2]))
    return wt(w1), wt(w2), bnv


# flat positions of the plane interior (row h, col c) -> DATA0 + h*PITCH + c
_INT_COLS = (DATA0 + (np.arange(H)[:, None] * PITCH + np.arange(W))).ravel()


def _in_maps(x, w1t, w2t, bnv):
    """Per-core input dicts: xq1 = sign(x) packed into the shared-pad fp8
    plane layout [p, b, kt, NPAD]; x = bf16 residual in [p, b, kt, hw]."""
    maps = []
    for c in range(N_CORES):
        xs = x[c * BL:(c + 1) * BL]                       # [BL, C, H, W]
        xh = np.ascontiguousarray(
            xs.reshape(BL, KT, P, H * W).transpose(2, 0, 1, 3))
        v = np.sign(xh)                                   # [P, BL, KT, H*W]
        plane = np.zeros((P, BL, KT, NPAD), np.float32)
        plane[:, :, :, _INT_COLS] = v
        xq = plane.astype(mybir.dt.np(FP8))
        maps.append({"xq1": xq, "x": xh.astype(mybir.dt.np(FP8)),
                     "w1t": w1t, "w2t": w2t, "bnv": bnv})
    return maps


def kernel(x, w1, gamma1, beta1, mean1, var1,
           w2, gamma2, beta2, mean2, var2):
    x = np.asarray(x, dtype=np.float32)
    w1t, w2t, bnv = _prep(
        np.asarray(w1), np.asarray(w2),
        np.asarray(gamma1), np.asarray(beta1), np.asarray(mean1), np.asarray(var1),
        np.asarray(gamma2), np.asarray(beta2), np.asarray(mean2), np.asarray(var2),
    )

    nc = _build()
    in_maps = _in_maps(x, w1t, w2t, bnv)

    res = run_bass_kernel_spmd(nc, in_maps, core_ids=list(range(N_CORES)))
    out = np.concatenate([r["out"] for r in res.results], axis=0)
    return out.astype(np.float32)

